# revision 1
# baseline (speedup 1.0000x reference)
"""Trainium2 Bass kernel for Conv2d_NN (k-NN gather + grouped conv1d).

Shapes (hardcoded): x (32, 32, 96, 96) f32, conv_w (256, 128, 9) f32,
conv_b (256,) f32 -> out (32, 64, 96, 96) f32.

Strategy: data-parallel over batch across 8 NeuronCores (4 batches/core).
Per batch on device (tokens N=2304, features D=128 after host pixel-unshuffle):
  - scores = x2^T @ x2 - 0.5*||x2_j||^2 via PE matmuls (n_sq folded in as a
    1-row augmented matmul; self-match excluded with a -BIG*I diagonal matmul)
  - ACT evacuates PSUM scores to SBUF
  - DVE vector.max / vector.max_index give each token's top-8 neighbors
    (self is always rank 0 and needs no gather)
  - a 2-hop DMA shuffle builds the wrapped index layout for gpsimd.ap_gather,
    which gathers neighbor feature columns
  - conv1d == 9 accumulating 128x128 matmuls per output half; ACT adds bias +
    ReLU; DMA writes (b, 256, 2304); host does the pixel-shuffle back.
"""

import sys

for _p in ("/opt/trn_rl_repo",):
    if _p not in sys.path:
        sys.path.insert(0, _p)

import numpy as np

import concourse.bass as bass
import concourse.mybir as mybir
import concourse.tile as tile
from concourse import bacc, bass_utils

# Problem constants
B, C_IN, C_OUT, H, W = 32, 32, 64, 96, 96
S = 2
K = 9
D = C_IN * S * S            # 128
D_OUT = C_OUT * S * S       # 256
N = (H // S) * (W // S)     # 2304
NCORES = 8
BPC = B // NCORES           # 4 batches per core

P = 128                     # partitions / m-tile size
NT = N // P                 # 18 m-tiles
CHUNK = 512                 # psum bank = 512 f32
CHUNKS = [(c, min(CHUNK, N - c)) for c in range(0, N, CHUNK)]  # 4x512 + 256
BIG = 1.0e30
CONV_GROUP = 4              # m-tiles per conv group (512 tokens)

_cache = {}


def _build_kernel(bpc=BPC, nt=NT):
    key = ("nc", bpc, nt)
    if key in _cache:
        return _cache[key], None

    nc = bacc.Bacc("TRN2", target_bir_lowering=False, debug=False)

    f32 = mybir.dt.float32
    u16 = mybir.dt.uint16
    i16 = mybir.dt.int16

    # I/O
    x2_d = nc.dram_tensor("x2", [bpc, D, N], f32, kind="ExternalInput")
    wt_d = nc.dram_tensor("wt", [D, K, 2, P], f32, kind="ExternalInput")
    bias_d = nc.dram_tensor("bias", [P, 2], f32, kind="ExternalInput")
    ident_d = nc.dram_tensor("ident", [P, P], f32, kind="ExternalInput")
    negbig_d = nc.dram_tensor("negbig", [P, P], f32, kind="ExternalInput")
    ones1_d = nc.dram_tensor("ones1", [1, P], f32, kind="ExternalInput")
    neghalf_d = nc.dram_tensor("neghalf", [P, 1], f32, kind="ExternalInput")
    out_d = nc.dram_tensor("out", [bpc, D_OUT, N], f32, kind="ExternalOutput")

    with tile.TileContext(nc) as tc:
        import contextlib

        with contextlib.ExitStack() as ctx:
            const_pool = ctx.enter_context(tc.tile_pool(name="consts", bufs=1))
            x2_pool = ctx.enter_context(tc.tile_pool(name="x2", bufs=2))
            sq_pool = ctx.enter_context(tc.tile_pool(name="sq", bufs=1))
            nsq_pool = ctx.enter_context(tc.tile_pool(name="nsq", bufs=2))
            scores_pool = ctx.enter_context(tc.tile_pool(name="scores", bufs=3))
            mx_pool = ctx.enter_context(tc.tile_pool(name="mx", bufs=4))
            widx_pool = ctx.enter_context(tc.tile_pool(name="widx", bufs=4))
            g_pool = ctx.enter_context(tc.tile_pool(name="g", bufs=2))
            outs_pool = ctx.enter_context(tc.tile_pool(name="outs", bufs=4))
            psum_pool = ctx.enter_context(
                tc.tile_pool(name="psum", bufs=1, space="PSUM")
            )
            psum_conv_pool = ctx.enter_context(
                tc.tile_pool(name="psumc", bufs=2, space="PSUM")
            )
            psum_nsq_pool = ctx.enter_context(
                tc.tile_pool(name="psumn", bufs=1, space="PSUM")
            )
            dram_pool = ctx.enter_context(
                tc.tile_pool(name="stage", bufs=4, space="DRAM")
            )

            # constants, loaded once
            wt_s = const_pool.tile([D, K * 2 * P], f32, tag="wt")
            nc.sync.dma_start(wt_s[:], wt_d.ap().rearrange("d k h c -> d (k h c)"))
            wt_v = wt_s[:].rearrange("d (k h c) -> d k h c", k=K, h=2, c=P)
            bias_s = const_pool.tile([P, 2], f32, tag="bias")
            nc.sync.dma_start(bias_s[:], bias_d.ap())
            ident_s = const_pool.tile([P, P], f32, tag="ident")
            nc.sync.dma_start(ident_s[:], ident_d.ap())
            negbig_s = const_pool.tile([P, P], f32, tag="negbig")
            nc.sync.dma_start(negbig_s[:], negbig_d.ap())
            ones1_s = const_pool.tile([1, P], f32, tag="ones1")
            nc.sync.dma_start(ones1_s[:], ones1_d.ap())
            neghalf_s = const_pool.tile([P, 1], f32, tag="neghalf")
            nc.sync.dma_start(neghalf_s[:], neghalf_d.ap())

            for b in range(bpc):
                x2 = x2_pool.tile([D, N], f32)
                nc.sync.dma_start(x2[:], x2_d.ap()[b])

                # n_sq: sq = x2*x2 on ACT, then ones^T @ sq on PE -> -0.5*nsq
                sq = sq_pool.tile([D, N], f32)
                nc.scalar.square(sq[:], x2[:])
                nsqh = nsq_pool.tile([1, N], f32)
                for c0, w in CHUNKS:
                    pn = psum_nsq_pool.tile([1, CHUNK], f32, tag="pnsq")
                    nc.tensor.matmul(
                        pn[:1, :w], lhsT=neghalf_s[:], rhs=sq[:, c0 : c0 + w],
                        start=True, stop=True,
                    )
                    nc.scalar.copy(nsqh[:1, c0 : c0 + w], pn[:1, :w])

                # conv group state
                g_group = None
                group_tiles = 0
                group_start = 0

                for mt in range(nt):
                    m0 = mt * P
                    # ---- scores matmuls into PSUM [128, 2304] ----
                    scp = psum_pool.tile([P, N], f32, tag="scores")
                    diag_chunk = m0 // CHUNK
                    for ci, (c0, w) in enumerate(CHUNKS):
                        nc.tensor.matmul(
                            scp[:, c0 : c0 + w],
                            lhsT=x2[:, m0 : m0 + P],
                            rhs=x2[:, c0 : c0 + w],
                            start=True, stop=False,
                        )
                    # self-exclusion: scores[p, m0+p] -= BIG
                    nc.tensor.matmul(
                        scp[:, m0 : m0 + P],
                        lhsT=negbig_s[:],
                        rhs=ident_s[:],
                        start=False, stop=False,
                    )
                    # -0.5*nsq broadcast row (1-row augmented matmul), closes groups
                    for ci, (c0, w) in enumerate(CHUNKS):
                        nc.tensor.matmul(
                            scp[:, c0 : c0 + w],
                            lhsT=ones1_s[:],
                            rhs=nsqh[:1, c0 : c0 + w],
                            start=False, stop=True,
                        )

                    # ---- evacuate PSUM -> SBUF (ACT), halves ----
                    scores = scores_pool.tile([P, N], f32)
                    half = N // 2
                    nc.scalar.copy(scores[:, :half], scp[:, :half])
                    nc.scalar.copy(scores[:, half:], scp[:, half:])

                    # ---- top-8 on DVE ----
                    mx8 = mx_pool.tile([P, 8], f32, tag="mx8")
                    nc.vector.max(out=mx8[:], in_=scores[:])
                    midx = mx_pool.tile([P, 8], u16, tag="midx")
                    nc.vector.max_index(midx[:], mx8[:], scores[:])

                    # ---- 2-hop DMA shuffle to wrapped gather-index layout ----
                    # staging[r*64 + u*8 + (k-1)] = midx[u*16+r, k-1]
                    stage_t = dram_pool.tile([1, 1024], u16)
                    st_dst = stage_t[:].rearrange(
                        "a (r u k) -> a u r k", r=16, u=8, k=8
                    ).squeeze(0)
                    nc.sync.dma_start(st_dst, midx[:])
                    # widx[16c+r, c2] = staging[r*64 + c2]  (c replicated via 0-step)
                    widx = widx_pool.tile([P, 64], i16)
                    st_src = (
                        stage_t[:]
                        .rearrange("a (r c2) -> a r c2", r=16, c2=64)
                        .unsqueeze(1)
                        .broadcast_to([1, 8, 16, 64])
                        .bitcast(i16)
                        .squeeze(0)
                    )
                    nc.sync.dma_start(widx[:], st_src)

                    # ---- gather neighbors k=1..8 on gpsimd ----
                    if group_tiles == 0:
                        gt = min(CONV_GROUP, nt - mt)
                        g_group = g_pool.tile([D, CONV_GROUP * 1024], f32)
                        group_start = m0
                        group_len = gt
                    nc.gpsimd.ap_gather(
                        g_group[:, group_tiles * 1024 : (group_tiles + 1) * 1024],
                        x2[:],
                        widx[:],
                        channels=P,
                        num_elems=N,
                        d=1,
                        num_idxs=1024,
                    )
                    group_tiles += 1

                    # ---- conv when group complete ----
                    if group_tiles == group_len:
                        gtok = group_tiles * P
                        gv = g_group[:, : group_tiles * 1024].rearrange(
                            "d (mt u k r) -> d mt u k r", mt=group_tiles, u=8, k=8, r=16
                        )
                        for h in range(2):
                            cp = psum_conv_pool.tile([P, CHUNK], f32, tag="pconv")
                            # k = 0: self columns, no gather needed
                            nc.tensor.matmul(
                                cp[:, :gtok],
                                lhsT=wt_v[:, 0, h, :],
                                rhs=x2[:, group_start : group_start + gtok],
                                start=True, stop=False,
                            )
                            for k in range(1, K):
                                nc.tensor.matmul(
                                    cp[:, :gtok],
                                    lhsT=wt_v[:, k, h, :],
                                    rhs=gv[:, :, :, k - 1, :],
                                    start=False, stop=(k == K - 1),
                                )
                            o_s = outs_pool.tile([P, CHUNK], f32)
                            nc.scalar.activation(
                                o_s[:, :gtok], cp[:, :gtok],
                                mybir.ActivationFunctionType.Relu,
                                bias=bias_s[:, h : h + 1],
                            )
                            nc.sync.dma_start(
                                out_d.ap()[b, h * P : (h + 1) * P,
                                           group_start : group_start + gtok],
                                o_s[:, :gtok],
                            )
                        group_tiles = 0

    nc.compile()
    _cache[key] = nc
    return nc, None


def _host_inputs(x, conv_w, conv_b):
    """Shared per-core constant inputs + per-core x2 slices."""
    x = np.ascontiguousarray(x, dtype=np.float32)
    b = x.shape[0]
    x1 = (
        x.reshape(b, C_IN, H // S, S, W // S, S)
        .transpose(0, 1, 3, 5, 2, 4)
        .reshape(b, D, N)
    )
    wt = np.ascontiguousarray(
        conv_w.reshape(2, P, D, K).transpose(2, 3, 0, 1), dtype=np.float32
    )  # [D, K, 2, P]; conv_w is (256,128,9) -> (2,128half) x d x k
    bias = np.ascontiguousarray(
        conv_b.reshape(2, P).transpose(1, 0), dtype=np.float32
    )  # [P, 2]
    ident = np.eye(P, dtype=np.float32)
    negbig = (-BIG * np.eye(P)).astype(np.float32)
    ones1 = np.ones((1, P), dtype=np.float32)
    neghalf = np.full((P, 1), -0.5, dtype=np.float32)
    return x1, dict(
        wt=wt, bias=bias, ident=ident, negbig=negbig, ones1=ones1, neghalf=neghalf
    )


def kernel(x, conv_w, conv_b):
    nc, _ = _build_kernel()
    x1, consts = _host_inputs(x, conv_w, conv_b)
    in_maps = []
    for c in range(NCORES):
        m = dict(consts)
        m["x2"] = np.ascontiguousarray(x1[c * BPC : (c + 1) * BPC])
        in_maps.append(m)
    res = bass_utils.run_bass_kernel_spmd(nc, in_maps, core_ids=list(range(NCORES)))
    outs = np.concatenate([r["out"] for r in res.results], axis=0)  # [B, 256, N]
    # pixel shuffle back: channel dim = (co, sy, sx); token = (h, w)
    o = outs.reshape(B, C_OUT, S, S, H // S, W // S)
    o = o.transpose(0, 1, 4, 2, 5, 3).reshape(B, C_OUT, H, W)
    return np.ascontiguousarray(o, dtype=np.float32)

